# revision 4
# baseline (speedup 1.0000x reference)
"""Trainium2 Bass kernel for nn_EntropyLoss (retrieval_knn).

Computes var([E(f1)-E(f0), E(f2)-E(f1)], ddof=1) where
E(f) = log(1 + sum_b sum_i r_ball[b, i]) and r_ball[b, i] is the K-th
nearest-neighbor distance (K = C//10 = 51, i.e. 52nd smallest including
the self-distance 0) among the C=512 channel vectors (dim H*W = 4096)
of sample b.

Strategy (8 NeuronCores, data-parallel over the 48 (tensor, sample)
units, 6 units per core):
  host:   pre-transpose each unit to X^T [4096, 512] in the PE-friendly
          [128, 32, 512] chunk layout, cast to fp16, and precompute
          chat[c] = fp16(2048 - ||x_c||^2 / 2)  (values ~0 +- 45, so the
          fp16 rounding eps is ~1e-2).
  device: per 128-row block, PSUM accumulates the SYMMETRIC ranking
          proxy mt = G + chat_i + chat_j via 2 K=1 bias matmuls (ones^T
          (x) chat adds chat_j; chat (x) ones^T adds chat_i) plus 32
          fp16 Gram k-chunk matmuls.  d2_ij = 8192 + 2eps_i + 2eps_j -
          2 mt_ij (the sq terms cancel), so per row the 52nd-smallest d2
          corresponds to the 52nd-largest mt.  Act copies mt PSUM->SBUF
          fp16.  The 52nd-largest value is then found by T=13 rounds of
          BISECTION COUNTING: count_j(mt > t) per row via ONE fused DVE
          tensor_scalar (is_gt + accumulate; runs in 4x DVE perf mode on
          fp16 ~3x faster than max8) or GPSIMD tensor_scalar or Act
          activation(Sign, bias=-t, accum_out) (2*cnt-512), with the
          per-row thresholds of a whole 4-block unit updated by two tiny
          [128,4] ops per round.  Counting is spread across DVE, Act and
          GPSIMD by assigning whole units to engines, which takes the
          13-pass-per-block max8/match_replace selection (the old DVE
          bottleneck, ~237 us) off the critical path; the PE matmuls
          (~170 us) become the bound.
  host:   d2 = 8192 + 2 eps_i - 2 est, r = sqrt(max(d2, 0)), then the
          scalar log/var tail in fp64.  Bisection grid [-127.997,
          384.003), final estimate = bracket midpoint (width 512/2^13 =
          0.0625); grid offset .003 avoids exact fp16 ties.
"""
import sys

for _p in ("/opt/trn_rl_repo", "/root/.axon_site/_ro/trn_rl_repo"):
    if _p not in sys.path:
        sys.path.insert(0, _p)

import numpy as np

from concourse import bacc, mybir
from concourse.tile import TileContext
from concourse.bass_utils import run_bass_kernel_spmd
from concourse.alu_op_type import AluOpType

B, C, H, W = 16, 512, 64, 64
D = H * W  # 4096
K = C // 10  # 51 -> want 52nd smallest distance per row
RANK = K + 1  # 52
N_CORES = 8
N_TENSORS = 3
UNITS = N_TENSORS * B  # 48
UPC = UNITS // N_CORES  # units per core = 6
KCHUNKS = D // 128  # 32
RBLK = C // 128  # 4 row blocks per unit
NBLK = UPC * RBLK  # 24 blocks per core

T_ITER = 13  # bisection rounds; final bracket width 512/2^13 = 0.0625
LO = -127.997  # grid offset .003 avoids exact fp16/grid ties
RNG = 512.0
TRY0 = LO + RNG / 2.0  # first test threshold

# unit -> counting engine. Act is slower per pass, so it gets the
# earliest-produced unit; DVE (4x fused tensor_scalar) takes the rest.
# (GPSIMD can't help: TensorScalarPtr is not a valid Pool opcode on CoreV3.)
UNIT_ENGINE = ("act", "dve", "dve", "dve", "dve", "dve")

TRACE = False  # test.py flips this for profiling
_LAST = {}  # debug stash

DMA_SPLIT = 4  # xt DMAs per sample (lets PE start on the first chunk early)


def _build_program(repeat=1, ablate=(), loop_n=None):
    """ablate: subset of {"sel", "mm", "dma"} for timing ablations.
    loop_n: if set, wrap the whole pipeline in a hardware For_i loop of
    that many iterations (device-side repetition for timing)."""
    nc = bacc.Bacc("TRN2", target_bir_lowering=False, debug=False)

    xt_d = nc.dram_tensor(
        "xt", [UPC, 128, KCHUNKS * C], mybir.dt.float16, kind="ExternalInput"
    )
    # cc[s, j] = fp16(2048 - sq[s, j]/2) = chat: folded into the Gram matmul
    # as TWO K=1 accumulation rows/cols so PSUM holds mt = G + chat_i + chat_j.
    cc_d = nc.dram_tensor("cc", [UPC, C], mybir.dt.float16, kind="ExternalInput")
    msel_d = nc.dram_tensor(
        "msel", [128, NBLK], mybir.dt.float32, kind="ExternalOutput"
    )

    kper = KCHUNKS // DMA_SPLIT  # k-chunks per DMA piece
    xt_view = xt_d.ap().rearrange(
        "s p (d k c) -> s p d k c", d=DMA_SPLIT, k=kper
    )

    Sign = mybir.ActivationFunctionType.Sign

    with TileContext(nc) as tc:
        with (
            tc.tile_pool(name="xpool", bufs=2 * DMA_SPLIT) as xpool,
            tc.tile_pool(name="consts", bufs=1) as consts,
            tc.tile_pool(name="mpool", bufs=NBLK) as mpool,
            tc.tile_pool(name="scr", bufs=2) as scrpool,
            tc.tile_pool(name="small", bufs=2) as small,
            tc.tile_pool(name="gps", bufs=6, space="PSUM") as gps,
        ):
            ones_row = consts.tile([1, 128], mybir.dt.float16)
            nc.vector.memset(ones_row, 1.0)
            ones_c = consts.tile([1, C], mybir.dt.float16)
            nc.vector.memset(ones_c, 1.0)
            msel = consts.tile([128, NBLK], mybir.dt.float32)
            # all 6 samples' chat rows in one partition-0 tile, one DMA
            cc_all = consts.tile([1, UPC * C], mybir.dt.float16)
            nc.sync.dma_start(
                out=cc_all, in_=cc_d.ap().rearrange("s c -> (s c)").unsqueeze(0)
            )

            def count_group(s, m4):
                """Bisection counting for unit s (4 row blocks m4)."""
                eng = UNIT_ENGINE[s]
                upd = nc.vector
                sgn = -1.0 if eng == "act" else 1.0  # act tracks -t
                thr = -408.5 if eng == "act" else (RANK - 0.5)
                nrounds = 1 if "sel" in ablate else T_ITER

                try_t = small.tile([128, RBLK], mybir.dt.float32, tag=f"try{s}")
                nc.vector.memset(try_t, sgn * TRY0)
                for k in range(1, nrounds + 1):
                    dk = RNG / (2.0 ** k)
                    cnt = small.tile([128, RBLK], mybir.dt.float32, tag=f"cnt{s}")
                    for i in range(RBLK):
                        tcol = try_t[:, i : i + 1]
                        ccol = cnt[:, i : i + 1]
                        if eng == "act":
                            scr = scrpool.tile(
                                [128, C], mybir.dt.float16, tag="scr_a"
                            )
                            nc.scalar.activation(
                                out=scr, in_=m4[i], func=Sign,
                                bias=tcol, scale=1.0, accum_out=ccol,
                            )
                        else:
                            scr = scrpool.tile(
                                [128, C], mybir.dt.float16, tag="scr_d"
                            )
                            nc.vector.tensor_scalar(
                                out=scr, in0=m4[i], scalar1=tcol, scalar2=None,
                                op0=AluOpType.is_gt, op1=AluOpType.add,
                                accum_out=ccol,
                            )
                    # u = (cnt >= thr) * (sgn*dk);  try' = u + (-sgn*dk/2) + try
                    u = small.tile([128, RBLK], mybir.dt.float32, tag=f"u{s}")
                    upd.tensor_scalar(
                        out=u, in0=cnt, scalar1=thr, scalar2=sgn * dk,
                        op0=AluOpType.is_ge, op1=AluOpType.mult,
                    )
                    if k == nrounds:
                        out_t = msel[:, s * RBLK : (s + 1) * RBLK]
                    else:
                        out_t = small.tile(
                            [128, RBLK], mybir.dt.float32, tag=f"try{s}"
                        )
                    upd.scalar_tensor_tensor(
                        out=out_t, in0=u, scalar=-sgn * dk / 2.0, in1=try_t,
                        op0=AluOpType.add, op1=AluOpType.add,
                    )
                    try_t = out_t

            def pipeline_body(_iv=None):
                xparts_cached = None
                for s in range(UPC):
                    if "dma" in ablate and xparts_cached is not None:
                        xparts = xparts_cached
                    else:
                        xparts = []
                        for d in range(DMA_SPLIT):
                            xp = xpool.tile(
                                [128, kper, C], mybir.dt.float16, tag="xts"
                            )
                            nc.sync.dma_start(out=xp, in_=xt_view[s, :, d])
                            xparts.append(xp)
                        xparts_cached = xparts

                    cc_s = cc_all[:, s * C : (s + 1) * C]

                    m4 = []
                    for I in range(RBLK):
                        g_ps = gps.tile([128, C], mybir.dt.float32, tag="g")
                        # two K=1 bias rows: mt += chat_j (cols) + chat_i (rows)
                        nc.tensor.matmul(
                            out=g_ps, lhsT=ones_row, rhs=cc_s,
                            start=True, stop=False,
                        )
                        nc.tensor.matmul(
                            out=g_ps,
                            lhsT=cc_all[:, s * C + 128 * I : s * C + 128 * (I + 1)],
                            rhs=ones_c,
                            start=False, stop=False,
                        )
                        nkc = 1 if "mm" in ablate else KCHUNKS
                        for k in range(nkc):
                            xp = xparts[k // kper]
                            kk = k % kper
                            nc.tensor.matmul(
                                out=g_ps,
                                lhsT=xp[:, kk, 128 * I : 128 * (I + 1)],
                                rhs=xp[:, kk, :],
                                start=False,
                                stop=(k == nkc - 1),
                            )
                        m = mpool.tile([128, C], mybir.dt.float16, tag="m")
                        nc.scalar.copy(out=m, in_=g_ps)
                        m4.append(m)
                    count_group(s, m4)

            if loop_n is not None:
                with tc.For_i(0, loop_n, 1) as _iv:
                    pipeline_body(_iv)
            else:
                for _rep in range(repeat):
                    pipeline_body()

            nc.sync.dma_start(out=msel_d.ap(), in_=msel)

    nc.compile()
    return nc


_PROGRAM = None


def kernel(feat0, feat1, feat2):
    global _PROGRAM
    feats = np.stack(
        [np.asarray(f).reshape(B, C, D) for f in (feat0, feat1, feat2)]
    ).reshape(UNITS, C, D)

    # sq in fp64 (host); chat = fp16(2048 - sq/2) enters the Gram as two K=1
    # bias matmuls so PSUM holds mt = G + chat_i + chat_j directly
    sq64 = np.einsum(
        "ucd,ucd->uc", feats, feats, dtype=np.float64, casting="safe"
    )
    chat16 = (2048.0 - sq64 / 2.0).astype(np.float16)
    eps = chat16.astype(np.float64) - (2048.0 - sq64 / 2.0)

    # X^T in [128, 32, 512] chunk layout, fp16
    # xt[u, p, k, c] = X[c, 128k + p]
    xt = np.ascontiguousarray(
        feats.astype(np.float16)
        .transpose(0, 2, 1)  # [U, D, C]
        .reshape(UNITS, KCHUNKS, 128, C)
        .transpose(0, 2, 1, 3)  # [U, 128, K, C]
        .reshape(UNITS, 128, KCHUNKS * C)
    )

    if _PROGRAM is None:
        _PROGRAM = _build_program()
    nc = _PROGRAM
    in_maps = [
        {
            "xt": xt[c * UPC : (c + 1) * UPC],
            "cc": chat16[c * UPC : (c + 1) * UPC],
        }
        for c in range(N_CORES)
    ]
    out = run_bass_kernel_spmd(
        nc, in_maps, core_ids=list(range(N_CORES)), trace=TRACE
    )
    _LAST.clear()
    _LAST["results"] = out

    # msel[p, s*4 + I] = bisection estimate of the 52nd-largest mt of row
    # (I*128 + p) of unit s (negated for act-counted units)
    est = np.empty((UNITS, C), dtype=np.float64)
    for c in range(N_CORES):
        sel = out.results[c]["msel"].astype(np.float64).reshape(128, UPC, RBLK)
        for s in range(UPC):
            v = sel[:, s, :]
            if UNIT_ENGINE[s] == "act":
                v = -v
            est[c * UPC + s] = v.transpose(1, 0).reshape(C)

    # d2 = 8192 + 2 eps_i - 2 mt52   (+2 eps_j* ~ 1e-2, ignored)
    d2 = 8192.0 + 2.0 * eps - 2.0 * est
    r = np.sqrt(np.clip(d2, 0.0, None))  # [UNITS, C]
    _LAST["r"] = r
    sums = r.reshape(N_TENSORS, B * C).sum(axis=1)
    e = np.log(sums + 1.0)
    deltas = np.array([e[1] - e[0], e[2] - e[1]])
    var = deltas.var(ddof=1)
    return np.asarray(var, dtype=np.float32)


# revision 17
# speedup vs baseline: 1.2775x; 1.2775x over previous
"""Trainium2 Bass kernel for nn_EntropyLoss (retrieval_knn).

Computes var([E(f1)-E(f0), E(f2)-E(f1)], ddof=1) where
E(f) = log(1 + sum_b sum_i r_ball[b, i]) and r_ball[b, i] is the K-th
nearest-neighbor distance (K = C//10 = 51, i.e. 52nd smallest including
the self-distance 0) among the C=512 channel vectors (dim H*W = 4096)
of sample b.

Strategy (8 NeuronCores, data-parallel over the 48 (tensor, sample)
units, 6 units per core):
  host:   pre-transpose each unit to X^T [4096, 512] in the PE-friendly
          [128, 32, 512] chunk layout, cast to fp16, and precompute
          chat[c] = fp16(2048 - ||x_c||^2 / 2)  (values ~0 +- 45, so the
          fp16 rounding eps is ~1e-2).
  device: per 128-row block, PSUM accumulates the SYMMETRIC ranking
          proxy mt = G + chat_i + chat_j via 2 K=1 bias matmuls (ones^T
          (x) chat adds chat_j; chat (x) ones^T adds chat_i) plus 32
          fp16 Gram k-chunk matmuls.  d2_ij = 8192 + 2eps_i + 2eps_j -
          2 mt_ij (the sq terms cancel), so per row the 52nd-smallest d2
          corresponds to the 52nd-largest mt.  Act copies mt PSUM->SBUF
          fp16.  The 52nd-largest value is then found by T=13 rounds of
          BISECTION COUNTING: count_j(mt > t) per row via ONE fused DVE
          tensor_scalar (is_gt + accumulate; runs in 4x DVE perf mode on
          fp16 ~3x faster than max8) or GPSIMD tensor_scalar or Act
          activation(Sign, bias=-t, accum_out) (2*cnt-512), with the
          per-row thresholds of a whole 4-block unit updated by two tiny
          [128,4] ops per round.  Counting is spread across DVE, Act and
          GPSIMD by assigning whole units to engines, which takes the
          13-pass-per-block max8/match_replace selection (the old DVE
          bottleneck, ~237 us) off the critical path; the PE matmuls
          (~170 us) become the bound.
  host:   d2 = 8192 + 2 eps_i - 2 est, r = sqrt(max(d2, 0)), then the
          scalar log/var tail in fp64.  Bisection grid [-127.997,
          384.003), final estimate = bracket midpoint (width 512/2^13 =
          0.0625); grid offset .003 avoids exact fp16 ties.
"""
import sys

for _p in ("/opt/trn_rl_repo", "/root/.axon_site/_ro/trn_rl_repo"):
    if _p not in sys.path:
        sys.path.insert(0, _p)

import numpy as np

from concourse import bacc, mybir
from concourse.tile import TileContext
from concourse.bass_utils import run_bass_kernel_spmd
from concourse.alu_op_type import AluOpType

B, C, H, W = 16, 512, 64, 64
D = H * W  # 4096
K = C // 10  # 51 -> want 52nd smallest distance per row
RANK = K + 1  # 52
N_CORES = 8
N_TENSORS = 3
UNITS = N_TENSORS * B  # 48
UPC = UNITS // N_CORES  # units per core = 6
KCHUNKS = D // 128  # 32
RBLK = C // 128  # 4 row blocks per unit
NBLK = UPC * RBLK  # 24 blocks per core

T_ITER = 13  # bisection rounds; final bracket width 512/2^13 = 0.0625
LO = -127.997  # grid offset .003 avoids exact fp16/grid ties
RNG = 512.0
TRY0 = LO + RNG / 2.0  # first test threshold

# unit -> counting engine. Act is slower per pass, so it gets the
# earliest-produced unit; DVE (4x fused tensor_scalar) takes the rest.
# (GPSIMD can't help: TensorScalarPtr is not a valid Pool opcode on CoreV3.)
UNIT_ENGINE = ("act", "dve", "dve", "dve", "dve", "dve")

TRACE = False  # test.py flips this for profiling
_LAST = {}  # debug stash

DMA_SPLIT = 4  # xt DMAs per sample (lets PE start on the first chunk early)

# mt is symmetric, so only block-columns J >= I are computed by matmul; the
# J < I part of each row block is a PE transpose (fp16, 128 cycles) of the
# already-copied SBUF tile of block J. Cuts PE cycles per unit from
# 34*4*512 to 34*1280 + 6*128 (~0.65x).
SYMM = True


def _build_program(repeat=1, ablate=(), loop_n=None):
    """ablate: subset of {"sel", "mm", "dma"} for timing ablations.
    loop_n: if set, wrap the whole pipeline in a hardware For_i loop of
    that many iterations (device-side repetition for timing)."""
    nc = bacc.Bacc("TRN2", target_bir_lowering=False, debug=False)

    xt_d = nc.dram_tensor(
        "xt", [UPC, 128, KCHUNKS * C], mybir.dt.float16, kind="ExternalInput"
    )
    # cc[s, j] = fp16(2048 - sq[s, j]/2) = chat: folded into the Gram matmul
    # as TWO K=1 accumulation rows/cols so PSUM holds mt = G + chat_i + chat_j.
    # cc2[0] = A = [chat; ones] (bias-matmul lhsT rows), cc2[1] = B =
    # [ones; chat] (rhs rows): one K=2 matmul adds chat_i + chat_j to PSUM.
    cc2_d = nc.dram_tensor(
        "cc2", [2, 2, UPC * C], mybir.dt.float16, kind="ExternalInput"
    )
    eye_d = nc.dram_tensor("eye", [128, 128], mybir.dt.float16, kind="ExternalInput")
    msel_d = nc.dram_tensor(
        "msel", [128, NBLK], mybir.dt.float32, kind="ExternalOutput"
    )

    kper = KCHUNKS // DMA_SPLIT  # k-chunks per DMA piece
    xt_view = xt_d.ap().rearrange(
        "s p (d k c) -> s p d k c", d=DMA_SPLIT, k=kper
    )

    Sign = mybir.ActivationFunctionType.Sign

    with TileContext(nc) as tc:
        with (
            tc.tile_pool(name="xpool", bufs=2 * DMA_SPLIT) as xpool,
            tc.tile_pool(name="consts", bufs=1) as consts,
            tc.tile_pool(name="mpool", bufs=NBLK) as mpool,
            tc.tile_pool(name="scr", bufs=2) as scrpool,
            tc.tile_pool(name="small", bufs=2) as small,
            tc.tile_pool(name="gps", bufs=5 if SYMM else 6, space="PSUM") as gps,
            tc.tile_pool(name="trs", bufs=2, space="PSUM") as trs,
        ):
            msel = consts.tile([128, NBLK], mybir.dt.float32)
            # A = [chat; ones] (lhsT), B = [ones; chat] (rhs): one DMA each
            cc_a = consts.tile([2, UPC * C], mybir.dt.float16)
            nc.sync.dma_start(out=cc_a, in_=cc2_d.ap()[0])
            cc_b = consts.tile([2, UPC * C], mybir.dt.float16)
            nc.sync.dma_start(out=cc_b, in_=cc2_d.ap()[1])
            eye = consts.tile([128, 128], mybir.dt.float16)
            nc.sync.dma_start(out=eye, in_=eye_d.ap())

            def count_group(s, m4):
                """Bisection counting for unit s (4 row blocks m4)."""
                eng = UNIT_ENGINE[s]
                upd = nc.vector
                sgn = -1.0 if eng == "act" else 1.0  # act tracks -t
                thr = -408.5 if eng == "act" else (RANK - 0.5)
                nrounds = 1 if "sel" in ablate else T_ITER

                try_t = small.tile([128, RBLK], mybir.dt.float32, tag=f"try{s}")
                nc.vector.memset(try_t, sgn * TRY0)
                for k in range(1, nrounds + 1):
                    dk = RNG / (2.0 ** k)
                    cnt = small.tile([128, RBLK], mybir.dt.float32, tag=f"cnt{s}")
                    for i in range(RBLK):
                        tcol = try_t[:, i : i + 1]
                        ccol = cnt[:, i : i + 1]
                        if eng == "act":
                            scr = scrpool.tile(
                                [128, C], mybir.dt.float16, tag="scr_a"
                            )
                            nc.scalar.activation(
                                out=scr, in_=m4[i], func=Sign,
                                bias=tcol, scale=1.0, accum_out=ccol,
                            )
                        else:
                            scr = scrpool.tile(
                                [128, C], mybir.dt.float16, tag="scr_d"
                            )
                            nc.vector.tensor_scalar(
                                out=scr, in0=m4[i], scalar1=tcol, scalar2=None,
                                op0=AluOpType.is_gt, op1=AluOpType.add,
                                accum_out=ccol,
                            )
                    # u = (cnt >= thr) * (sgn*dk);  try' = u + (-sgn*dk/2) + try
                    u = small.tile([128, RBLK], mybir.dt.float32, tag=f"u{s}")
                    upd.tensor_scalar(
                        out=u, in0=cnt, scalar1=thr, scalar2=sgn * dk,
                        op0=AluOpType.is_ge, op1=AluOpType.mult,
                    )
                    if k == nrounds:
                        out_t = msel[:, s * RBLK : (s + 1) * RBLK]
                    else:
                        out_t = small.tile(
                            [128, RBLK], mybir.dt.float32, tag=f"try{s}"
                        )
                    upd.scalar_tensor_tensor(
                        out=out_t, in0=u, scalar=-sgn * dk / 2.0, in1=try_t,
                        op0=AluOpType.add, op1=AluOpType.add,
                    )
                    try_t = out_t

            def pipeline_body(_iv=None):
                xparts_cached = None
                for s in range(UPC):
                    if "dma" in ablate and xparts_cached is not None:
                        xparts = xparts_cached
                    else:
                        xparts = []
                        for d in range(DMA_SPLIT):
                            xp = xpool.tile(
                                [128, kper, C], mybir.dt.float16, tag="xts"
                            )
                            nc.sync.dma_start(out=xp, in_=xt_view[s, :, d])
                            xparts.append(xp)
                        xparts_cached = xparts

                    m4 = []
                    for I in range(RBLK):
                        # direct part: block-columns J >= I (cols c0:512),
                        # written into the left w cols of a full-width bank
                        c0 = 128 * I if SYMM else 0
                        w = C - c0
                        g_full = gps.tile([128, C], mybir.dt.float32, tag="g")
                        g_ps = g_full[:, :w]
                        # one K=2 bias matmul: mt += chat_i (rows) + chat_j (cols)
                        nc.tensor.matmul(
                            out=g_ps,
                            lhsT=cc_a[:, s * C + 128 * I : s * C + 128 * (I + 1)],
                            rhs=cc_b[:, s * C + c0 : (s + 1) * C],
                            start=True, stop=False,
                        )
                        nkc = 1 if "mm" in ablate else KCHUNKS
                        for k in range(nkc):
                            xp = xparts[k // kper]
                            kk = k % kper
                            nc.tensor.matmul(
                                out=g_ps,
                                lhsT=xp[:, kk, 128 * I : 128 * (I + 1)],
                                rhs=xp[:, kk, c0:],
                                start=False,
                                stop=(k == nkc - 1),
                            )
                        m = mpool.tile([128, C], mybir.dt.float16, tag="m")
                        if SYMM and I > 0:
                            # block-columns J < I: transpose of block J's
                            # already-copied fp16 tile (mt is symmetric)
                            t_full = trs.tile(
                                [128, 128 * (RBLK - 1)], mybir.dt.float16,
                                tag="t",
                            )
                            t_ps = t_full[:, : 128 * I]
                            for J in range(I):
                                nc.tensor.transpose(
                                    out=t_ps[:, 128 * J : 128 * (J + 1)],
                                    in_=m4[J][:, 128 * I : 128 * (I + 1)],
                                    identity=eye,
                                )
                            nc.scalar.copy(out=m[:, :c0], in_=t_ps)
                        nc.scalar.copy(out=m[:, c0:], in_=g_ps)
                        m4.append(m)
                    count_group(s, m4)

            if loop_n is not None:
                with tc.For_i(0, loop_n, 1) as _iv:
                    pipeline_body(_iv)
            else:
                for _rep in range(repeat):
                    pipeline_body()

            nc.sync.dma_start(out=msel_d.ap(), in_=msel)

    nc.compile()
    return nc


_PROGRAM = None


def kernel(feat0, feat1, feat2):
    global _PROGRAM
    feats = np.stack(
        [np.asarray(f).reshape(B, C, D) for f in (feat0, feat1, feat2)]
    ).reshape(UNITS, C, D)

    # sq in fp64 (host); chat = fp16(2048 - sq/2) enters the Gram as two K=1
    # bias matmuls so PSUM holds mt = G + chat_i + chat_j directly
    sq64 = np.einsum(
        "ucd,ucd->uc", feats, feats, dtype=np.float64, casting="safe"
    )
    chat16 = (2048.0 - sq64 / 2.0).astype(np.float16)
    eps = chat16.astype(np.float64) - (2048.0 - sq64 / 2.0)

    # X^T in [128, 32, 512] chunk layout, fp16
    # xt[u, p, k, c] = X[c, 128k + p]
    xt = np.ascontiguousarray(
        feats.astype(np.float16)
        .transpose(0, 2, 1)  # [U, D, C]
        .reshape(UNITS, KCHUNKS, 128, C)
        .transpose(0, 2, 1, 3)  # [U, 128, K, C]
        .reshape(UNITS, 128, KCHUNKS * C)
    )

    if _PROGRAM is None:
        _PROGRAM = _build_program()
    nc = _PROGRAM
    eye = np.eye(128, dtype=np.float16)

    def _cc2(c):
        ch = chat16[c * UPC : (c + 1) * UPC].reshape(UPC * C)
        on = np.ones(UPC * C, dtype=np.float16)
        return np.stack([np.stack([ch, on]), np.stack([on, ch])])

    in_maps = [
        {
            "xt": xt[c * UPC : (c + 1) * UPC],
            "cc2": _cc2(c),
            "eye": eye,
        }
        for c in range(N_CORES)
    ]
    out = run_bass_kernel_spmd(
        nc, in_maps, core_ids=list(range(N_CORES)), trace=TRACE
    )
    _LAST.clear()
    _LAST["results"] = out

    # msel[p, s*4 + I] = bisection estimate of the 52nd-largest mt of row
    # (I*128 + p) of unit s (negated for act-counted units)
    est = np.empty((UNITS, C), dtype=np.float64)
    for c in range(N_CORES):
        sel = out.results[c]["msel"].astype(np.float64).reshape(128, UPC, RBLK)
        for s in range(UPC):
            v = sel[:, s, :]
            if UNIT_ENGINE[s] == "act":
                v = -v
            est[c * UPC + s] = v.transpose(1, 0).reshape(C)

    # d2 = 8192 + 2 eps_i - 2 mt52   (+2 eps_j* ~ 1e-2, ignored)
    d2 = 8192.0 + 2.0 * eps - 2.0 * est
    r = np.sqrt(np.clip(d2, 0.0, None))  # [UNITS, C]
    _LAST["r"] = r
    sums = r.reshape(N_TENSORS, B * C).sum(axis=1)
    e = np.log(sums + 1.0)
    deltas = np.array([e[1] - e[0], e[2] - e[1]])
    var = deltas.var(ddof=1)
    return np.asarray(var, dtype=np.float32)
